# revision 1
# baseline (speedup 1.0000x reference)
"""Trainium2 Bass kernel for nn_Capsule (EM capsule routing).

Strategy (8 NeuronCores, SPMD):
  - EM loop (3 iters): batch-parallel, 4 batch elements per core. The E-step
    Mahalanobis term is expanded so it becomes one matmul per n-chunk against a
    block-diagonal coefficient matrix (contraction = (b,2d)=128), producing
    logits in [n', (b,k)] layout so softmax-over-k is a free-axis segmented
    reduce. M-step moments (Sx, Sxx, N_k) come from one accumulating matmul
    with R as the stationary operand.
  - One AllToAll redistributes R from batch-sharding to k-sharding (1 MB/core).
  - Final einsum s[b,k,d] = sum_{n,e} W[n,k,d,e] R[b,n,k] x[b,n,e] is
    k-sharded (4 output capsules per core, W slice 8.4 MB). Contraction over n
    in chunks of 128, one accumulating matmul per (e, chunk); the moving
    operand y = R*x is built by DVE with free-dim broadcasts only.
  - Each core computes its own output k-slice; host concatenates.
"""

import numpy as np

import concourse.bass as bass
import concourse.bacc as bacc
import concourse.tile as tile
from concourse import mybir
from concourse.bass_utils import run_bass_kernel_spmd

F32 = mybir.dt.float32
B, N, K, D = 32, 2048, 32, 16
NC, BL, KL = 8, 4, 4
NCH = N // 128           # 16 n-chunks
LOG2PI = float(np.log(2.0 * np.pi))
EPS = 1e-7
ROUTINGS = 3
SLAB = 4                 # chunks per softmax slab (FD = 512)


def _ap(ref, offset_elems, dims):
    """Build an AP on ref's tensor with explicit [step, count] dims (elements)."""
    return bass.AP(tensor=ref.tensor, offset=ref.offset + offset_elems, ap=list(dims))


def _build_program():
    nc = bacc.Bacc("TRN2", target_bir_lowering=False)

    xstack = nc.dram_tensor("xstack", [128, N], F32, kind="ExternalInput")
    xmstep = nc.dram_tensor("xmstep", [N, 129], F32, kind="ExternalInput")
    coef0 = nc.dram_tensor("coef0", [128, 128], F32, kind="ExternalInput")
    ccr0 = nc.dram_tensor("ccr0", [128, 128], F32, kind="ExternalInput")
    xfe = nc.dram_tensor("xfe", [N, 16, 32], F32, kind="ExternalInput")
    wstack = nc.dram_tensor("wstack", [16, NCH, 128, 64], F32, kind="ExternalInput")
    ident = nc.dram_tensor("ident", [128, 128], F32, kind="ExternalInput")
    out_t = nc.dram_tensor("out_t", [32, 64], F32, kind="ExternalOutput")

    with tile.TileContext(nc) as tc:
        with (
            tc.tile_pool(name="per", bufs=1) as per,
            tc.tile_pool(name="cf", bufs=1) as cfpool,
            tc.tile_pool(name="soft", bufs=1) as soft,
            tc.tile_pool(name="sm", bufs=4) as sm,
            tc.tile_pool(name="yt", bufs=3) as ytpool,
            tc.tile_pool(name="lg", bufs=2, space="PSUM") as lgpool,
            tc.tile_pool(name="msps", bufs=1, space="PSUM") as mspool,
            tc.tile_pool(name="tps", bufs=3, space="PSUM") as tps,
            tc.tile_pool(name="feps", bufs=1, space="PSUM") as feps,
            tc.tile_pool(name="dram", bufs=1, space="DRAM") as dram,
        ):
            # ---------- persistent loads ----------
            xs = per.tile([128, N], F32)
            for ch in range(NCH):
                nc.sync.dma_start(out=xs[:, ch * 128:(ch + 1) * 128],
                                  in_=xstack[:, ch * 128:(ch + 1) * 128])
            xm = per.tile([128, NCH, 129], F32)
            xmr = xmstep[:].rearrange("(c p) f -> p c f", p=128)
            for ch in range(NCH):
                nc.sync.dma_start(out=xm[:, ch, :], in_=xmr[:, ch, :])
            idt = per.tile([128, 128], F32)
            nc.sync.dma_start(out=idt, in_=ident[:])
            cfb_in = per.tile([128, 128], F32)
            nc.sync.dma_start(out=cfb_in, in_=coef0[:])
            ccr_in = per.tile([128, 128], F32)
            nc.sync.dma_start(out=ccr_in, in_=ccr0[:])
            xf = per.tile([128, NCH, 16, 32], F32)
            xfr = xfe[:].rearrange("(c p) e b -> p c e b", p=128)
            for ch in range(NCH):
                nc.sync.dma_start(out=xf[:, ch], in_=xfr[:, ch])
            wst = per.tile([128, 16, NCH, 64], F32)
            wsr = wstack[:].rearrange("e c p f -> p e c f")
            for ch in range(NCH):
                nc.sync.dma_start(out=wst[:, :, ch, :], in_=wsr[:, :, ch, :])
            ones1 = per.tile([1, 128], F32)
            nc.vector.memset(ones1, 1.0)

            cfb_work = cfpool.tile([128, 128], F32)
            nc.gpsimd.memset(cfb_work, 0.0)
            ccr_work = cfpool.tile([128, 128], F32)

            Ptil = soft.tile([128, NCH, 128], F32)   # exp(logits+cconst)
            lgC = soft.tile([128, NCH, 128], F32)    # logits + cconst
            Zr = soft.tile([128, NCH, 4], F32)       # 1/Z per (n', ch, b)
            Rbuf = soft.tile([128, NCH, 128], F32)   # R, (b,k) col order
            Rlast = soft.tile([128, NCH, 128], F32)  # R final, (k,b)->4k+b col order

            R_loc = dram.tile([8, N, 16], F32)
            RGa = dram.tile([8, N, 16], F32)

            cc_C = float(np.exp(-np.log(float(N)) - (D / 2.0) * LOG2PI))

            for it in range(ROUTINGS):
                cfb = cfb_in if it == 0 else cfb_work
                ccr = ccr_in if it == 0 else ccr_work
                last = it == ROUTINGS - 1

                for s in range(NCH // SLAB):
                    chs = slice(s * SLAB, (s + 1) * SLAB)
                    lg_ps = lgpool.tile([128, SLAB, 128], F32)
                    for i in range(SLAB):
                        ch = s * SLAB + i
                        nc.tensor.matmul(
                            lg_ps[:, i, :],
                            xs[:, ch * 128:(ch + 1) * 128],
                            cfb,
                            start=True,
                            stop=True,
                        )
                    # evict logits PSUM->SBUF on ACT, add cconst on GPSIMD
                    # (keeps DVE free for the reduce/normalize chain)
                    lgE = sm.tile([128, SLAB, 128], F32)
                    nc.scalar.copy(lgE, lg_ps)
                    h = SLAB // 2
                    nc.vector.tensor_add(
                        lgC[:, s * SLAB:s * SLAB + h, :],
                        lgE[:, 0:h],
                        _ap(ccr[:], 0, [[128, 128], [0, h], [1, 128]]),
                    )
                    nc.gpsimd.tensor_add(
                        lgC[:, s * SLAB + h:(s + 1) * SLAB, :],
                        lgE[:, h:SLAB],
                        _ap(ccr[:], 0, [[128, 128], [0, SLAB - h], [1, 128]]),
                    )
                    nc.scalar.activation(
                        Ptil[:, chs, :], lgC[:, chs, :],
                        mybir.ActivationFunctionType.Exp,
                    )
                    # Z over k segments: view [p, (c b), k]
                    zt = sm.tile([128, SLAB, 4], F32)
                    nc.vector.reduce_sum(
                        zt.rearrange("p c b -> p (c b)"),
                        Ptil[:, chs, :].rearrange("p c (b k) -> p (c b) k", b=4),
                        axis=mybir.AxisListType.X,
                    )
                    nc.vector.reciprocal(Zr[:, chs, :].rearrange("p c b -> p (c b)"),
                                         zt.rearrange("p c b -> p (c b)"))
                    for eng, c0, c1 in ((nc.vector, 0, 3), (nc.gpsimd, 3, SLAB)):
                        sub = slice(s * SLAB + c0, s * SLAB + c1)
                        zr_b = _ap(Zr[:, sub, :], 0,
                                   [[NCH * 4, 128], [4, c1 - c0], [1, 4], [0, K]])
                        if not last:
                            # R in (b,k) col order for the M-step
                            eng.tensor_mul(
                                Rbuf[:, sub, :].rearrange("p c (b k) -> p c b k", b=4),
                                Ptil[:, sub, :].rearrange("p c (b k) -> p c b k", b=4),
                                zr_b,
                            )
                        else:
                            # final R written with permuted cols: col' = 4k + b
                            eng.tensor_mul(
                                Rlast[:, sub, :].rearrange("p c (k b) -> p c b k", k=K),
                                Ptil[:, sub, :].rearrange("p c (b k) -> p c b k", b=4),
                                zr_b,
                            )

                if last:
                    break

                # ---------- M-step ----------
                ms_ps = mspool.tile([128, 129], F32)
                for ch in range(NCH):
                    nc.tensor.matmul(
                        ms_ps, Rbuf[:, ch, :], xm[:, ch, :],
                        start=(ch == 0), stop=(ch == NCH - 1),
                    )

                rNk = sm.tile([128, 1], F32)
                nc.vector.reciprocal(rNk, ms_ps[:, 128:129])
                SxA = sm.tile([128, 16], F32)
                SxxA = sm.tile([128, 16], F32)
                for b in range(BL):
                    rows = slice(32 * b, 32 * b + 32)
                    nc.scalar.copy(SxA[rows, :], ms_ps[rows, 16 * b:16 * b + 16])
                    nc.scalar.copy(SxxA[rows, :], ms_ps[rows, 64 + 16 * b:64 + 16 * b + 16])
                mu = sm.tile([128, 16], F32)
                nc.vector.tensor_mul(mu, SxA, rNk.to_broadcast([128, 16]))
                ex2 = sm.tile([128, 16], F32)
                nc.vector.tensor_mul(ex2, SxxA, rNk.to_broadcast([128, 16]))
                musq = sm.tile([128, 16], F32)
                nc.vector.tensor_mul(musq, mu, mu)
                sig2 = sm.tile([128, 16], F32)
                nc.vector.tensor_sub(sig2, ex2, musq)
                a_r = sm.tile([128, 16], F32)
                nc.vector.reciprocal(a_r, sig2)
                bco = sm.tile([128, 16], F32)
                nc.vector.tensor_mul(bco, mu, a_r)

                coefsrc = sm.tile([128, 32], F32)
                nc.scalar.activation(coefsrc[:, 0:16], a_r,
                                     mybir.ActivationFunctionType.Copy, scale=-0.5)
                nc.scalar.copy(coefsrc[:, 16:32], bco)

                lns = sm.tile([128, 16], F32)
                nc.scalar.activation(lns, sig2, mybir.ActivationFunctionType.Ln)
                s1 = sm.tile([128, 1], F32)
                nc.vector.reduce_sum(s1, lns, axis=mybir.AxisListType.X)
                mb = sm.tile([128, 16], F32)
                nc.vector.tensor_mul(mb, mu, bco)
                s2 = sm.tile([128, 1], F32)
                nc.vector.reduce_sum(s2, mb, axis=mybir.AxisListType.X)
                s12 = sm.tile([128, 1], F32)
                nc.vector.tensor_add(s12, s1, s2)
                lnNkc = sm.tile([128, 1], F32)
                nc.scalar.activation(lnNkc, ms_ps[:, 128:129],
                                     mybir.ActivationFunctionType.Ln, scale=cc_C)
                cc = sm.tile([128, 1], F32)
                nc.scalar.activation(cc, s12, mybir.ActivationFunctionType.Identity,
                                     bias=lnNkc, scale=-0.5)

                # coefblock diagonal: transpose [128(b,k), 32(2d)] -> [32(2d), 128(b,k)]
                # then DMA each b's block onto the diagonal (DMA remaps partitions).
                cfT_ps = tps.tile([32, 128], F32, tag="tp")
                nc.tensor.transpose(cfT_ps, coefsrc, idt)
                cfT_sb = sm.tile([32, 128], F32)
                nc.scalar.copy(cfT_sb, cfT_ps)
                for b in range(BL):
                    rows = slice(32 * b, 32 * b + 32)
                    nc.gpsimd.dma_start(
                        out=cfb_work[rows, 32 * b:32 * b + 32],
                        in_=cfT_sb[:, rows],
                    )

                # cconst replicated: transpose [128,1]->[1,128], then ones-outer-product
                ccrow_ps = tps.tile([1, 128], F32, tag="tp")
                nc.tensor.transpose(ccrow_ps, cc, idt)
                ccrowS = sm.tile([1, 128], F32)
                nc.scalar.copy(ccrowS, ccrow_ps)
                ccr_ps = tps.tile([128, 128], F32, tag="tp")
                nc.tensor.matmul(ccr_ps, ones1, ccrowS, start=True, stop=True)
                nc.scalar.copy(ccr_work, ccr_ps)

            # ---------- redistribute R: batch-shard -> k-shard ----------
            for g in range(8):
                nc.sync.dma_start(
                    out=R_loc[g].rearrange("(c p) f -> p c f", p=128),
                    in_=Rlast[:, :, 16 * g:16 * g + 16],
                )
            nc.gpsimd.collective_compute(
                "AllToAll",
                mybir.AluOpType.bypass,
                replica_groups=[list(range(NC))],
                ins=[R_loc[:].opt()],
                outs=[RGa[:].opt()],
            )
            # Rfe[p, ch, kk, b] with b = 4*r + bl contiguous (32 cols per kk)
            Rfe = per.tile([128, NCH, 4, 32], F32)
            for r in range(8):
                rga_r = RGa[r].rearrange("(c p) (kk bl) -> p c kk bl", p=128, kk=4)
                for kk in range(4):
                    nc.sync.dma_start(
                        out=Rfe[:, :, kk, 4 * r:4 * r + 4],
                        in_=rga_r[:, :, kk, :],
                    )

            # ---------- final einsum, k-sharded ----------
            s_ps = feps.tile([64, 128], F32)
            for ch in range(NCH):
                yt = ytpool.tile([128, 16, 4, 8, 4], F32)
                # y[p,(e,kk,r,bl)] = R[p,(kk,r,bl)] * x[p,(e,b)]
                # split the y build across DVE (e<10) and GPSIMD (e>=10):
                # GPSIMD runs ~2x slower per element, so it gets 6/16 of e.
                ytv = yt.rearrange("p e kk r bl -> p e kk (r bl)")
                for eng, e0, e1 in ((nc.vector, 0, 11), (nc.gpsimd, 11, 16)):
                    ecnt = e1 - e0
                    eng.tensor_mul(
                        ytv[:, e0:e1],
                        _ap(Rfe[:, ch], 0,
                            [[NCH * 4 * 32, 128], [0, ecnt], [32, 4], [1, 32]]),
                        _ap(xf[:, ch], 32 * e0,
                            [[NCH * 16 * 32, 128], [32, ecnt], [0, 4], [1, 32]]),
                    )
                for e in range(16):
                    nc.tensor.matmul(
                        s_ps,
                        wst[:, e, ch, :],
                        yt[:, e].rearrange("p a b c -> p (a b c)"),
                        start=(ch == 0 and e == 0),
                        stop=(ch == NCH - 1 and e == 15),
                    )

            # ---------- squash + output ----------
            s_sb = sm.tile([64, 128], F32)
            nc.scalar.copy(s_sb, s_ps)
            sq = sm.tile([64, 32], F32)
            for kk in range(KL):
                rows = slice(16 * kk, 16 * kk + 16)
                nc.gpsimd.dma_start(out=sq[rows, :],
                                     in_=s_sb[rows, 32 * kk:32 * kk + 32])
            sqT_ps = tps.tile([32, 64], F32, tag="tp")
            nc.tensor.transpose(sqT_ps, sq, idt[0:64, 0:64])
            sqT = sm.tile([32, 64], F32)
            nc.scalar.copy(sqT, sqT_ps)
            ssq = sm.tile([32, 64], F32)
            nc.vector.tensor_mul(ssq, sqT, sqT)
            ss = sm.tile([32, 4], F32)
            nc.vector.reduce_sum(ss, ssq.rearrange("p (kk d) -> p kk d", kk=4),
                                 axis=mybir.AxisListType.X)
            eps_t = sm.tile([32, 1], F32)
            nc.vector.memset(eps_t, EPS)
            onep_t = sm.tile([32, 1], F32)
            nc.vector.memset(onep_t, 1.0 + EPS)
            nrm = sm.tile([32, 4], F32)
            nc.scalar.activation(nrm, ss, mybir.ActivationFunctionType.Sqrt, bias=eps_t)
            den = sm.tile([32, 4], F32)
            nc.scalar.activation(den, ss, mybir.ActivationFunctionType.Identity,
                                 bias=onep_t)
            rden = sm.tile([32, 4], F32)
            nc.vector.reciprocal(rden, den)
            fac = sm.tile([32, 4], F32)
            nc.vector.tensor_mul(fac, nrm, rden)
            outT = sm.tile([32, 64], F32)
            nc.vector.tensor_mul(
                outT.rearrange("p (kk d) -> p kk d", kk=4),
                sqT.rearrange("p (kk d) -> p kk d", kk=4),
                _ap(fac[:], 0, [[4, 32], [1, 4], [0, 16]]),
            )
            nc.sync.dma_start(out=out_t[:], in_=outT)

    nc.finalize()
    return nc


_PROG = None


def _get_program():
    global _PROG
    if _PROG is None:
        _PROG = _build_program()
    return _PROG


def _stage_inputs(x, W, mu0):
    x = np.asarray(x, np.float32)
    W = np.asarray(W, np.float32)
    mu0 = np.asarray(mu0, np.float32)
    I128 = np.eye(128, dtype=np.float32)
    # xfe[n, e, b] = x[b, n, e]  (global, shared by all cores)
    xfe = np.ascontiguousarray(x.transpose(1, 2, 0))

    in_maps = []
    for c in range(NC):
        xl = x[BL * c:BL * c + BL]          # [4, N, D]
        x2l = xl * xl
        mul = mu0[BL * c:BL * c + BL]       # [4, K, D]

        xstack = np.empty((128, N), np.float32)
        for b in range(BL):
            xstack[32 * b:32 * b + 16] = x2l[b].T
            xstack[32 * b + 16:32 * b + 32] = xl[b].T

        xmstep = np.empty((N, 129), np.float32)
        for b in range(BL):
            xmstep[:, 16 * b:16 * b + 16] = xl[b]
            xmstep[:, 64 + 16 * b:64 + 16 * b + 16] = x2l[b]
        xmstep[:, 128] = 1.0

        coef0 = np.zeros((128, 128), np.float32)
        for b in range(BL):
            coef0[32 * b:32 * b + 16, 32 * b:32 * b + 32] = -0.5
            coef0[32 * b + 16:32 * b + 32, 32 * b:32 * b + 32] = mul[b].T
        cconst = (np.log(1.0 / K)
                  - 0.5 * ((mul * mul).sum(-1) + D * LOG2PI))  # [4, K]
        ccr0 = np.broadcast_to(cconst.reshape(1, 128), (128, 128)).astype(np.float32)
        ccr0 = np.ascontiguousarray(ccr0)

        # wstack[e, ch, n', 16*kk+dd] = W[0, 128*ch+n', 4c+kk, dd, e]
        Wl = W[0][:, KL * c:KL * c + KL]    # [N, 4, D, E]
        wstack = np.ascontiguousarray(
            Wl.reshape(NCH, 128, KL, D, D).transpose(4, 0, 1, 2, 3)
            .reshape(16, NCH, 128, 64))

        in_maps.append({
            "xstack": xstack, "xmstep": xmstep, "coef0": coef0, "ccr0": ccr0,
            "xfe": xfe, "wstack": wstack, "ident": I128,
        })
    return in_maps


_RUNNER = None


def _get_runner():
    """Build (once) a jitted SPMD executor mirroring bass2jax.run_bass_via_pjrt
    so repeat calls reuse the compiled executable."""
    global _RUNNER
    if _RUNNER is not None:
        return _RUNNER
    import jax
    from jax.sharding import Mesh, PartitionSpec
    from jax.experimental.shard_map import shard_map
    from concourse import bass2jax, mybir as _mb

    nc = _get_program()
    bass2jax.install_neuronx_cc_hook()
    partition_name = nc.partition_id_tensor.name if nc.partition_id_tensor else None
    in_names, out_names, out_avals, zero_outs = [], [], [], []
    for alloc in nc.m.functions[0].allocations:
        if not isinstance(alloc, _mb.MemoryLocationSet):
            continue
        name = alloc.memorylocations[0].name
        if alloc.kind == "ExternalInput":
            if name != partition_name:
                in_names.append(name)
        elif alloc.kind == "ExternalOutput":
            shape = tuple(alloc.tensor_shape)
            dtype = _mb.dt.np(alloc.dtype)
            out_names.append(name)
            out_avals.append(jax.core.ShapedArray(shape, dtype))
            zero_outs.append(np.zeros(shape, dtype))
    n_params = len(in_names)
    n_outs = len(out_avals)
    all_names = in_names + out_names
    if partition_name is not None:
        all_names.append(partition_name)

    def _body(*args):
        operands = list(args)
        if partition_name is not None:
            operands.append(bass2jax.partition_id_tensor())
        outs = bass2jax._bass_exec_p.bind(
            *operands,
            out_avals=tuple(out_avals),
            in_names=tuple(all_names),
            out_names=tuple(out_names),
            lowering_input_output_aliases=(),
            sim_require_finite=True,
            sim_require_nnan=True,
            nc=nc,
        )
        return tuple(outs)

    global _MESH
    devices = jax.devices()[:NC]
    mesh = Mesh(np.asarray(devices), ("core",))
    _MESH = mesh
    in_specs = (PartitionSpec("core"),) * (n_params + n_outs)
    out_specs = (PartitionSpec("core"),) * n_outs
    sharded = jax.jit(
        shard_map(_body, mesh=mesh, in_specs=in_specs, out_specs=out_specs,
                  check_rep=False),
        keep_unused=True,
    )

    def run(in_maps):
        concat_in = [
            np.concatenate([np.asarray(in_maps[c][n]) for c in range(NC)], axis=0)
            for n in in_names
        ]
        concat_zeros = [
            np.zeros((NC * z.shape[0], *z.shape[1:]), z.dtype) for z in zero_outs
        ]
        out_arrs = sharded(*concat_in, *concat_zeros)
        return [
            {n: np.asarray(out_arrs[i]).reshape(NC, *out_avals[i].shape)[c]
             for i, n in enumerate(out_names)}
            for c in range(NC)
        ]

    run._sharded = (sharded, (in_names, zero_outs, NC))
    _RUNNER = run
    return run


def kernel(x, W, mu0):
    run = _get_runner()
    in_maps = _stage_inputs(x, W, mu0)
    results = run(in_maps)
    out = np.empty((B, K, D), np.float32)
    for c in range(NC):
        out[:, KL * c:KL * c + KL, :] = results[c]["out_t"].reshape(B, KL, D)
    kernel._last_in_maps = in_maps
    kernel._last_run = run
    kernel._last_sharded = run._sharded
    return out



# revision 2
# speedup vs baseline: 119.5128x; 119.5128x over previous
"""Trainium2 Bass kernel for nn_Capsule (EM capsule routing).

Strategy (8 NeuronCores, SPMD):
  - EM loop (3 iters): batch-parallel, 4 batch elements per core. The E-step
    Mahalanobis term is expanded so it becomes one matmul per n-chunk against a
    block-diagonal coefficient matrix (contraction = (b,2d)=128), producing
    logits in [n', (b,k)] layout so softmax-over-k is a free-axis segmented
    reduce. M-step moments (Sx, Sxx, N_k) come from one accumulating matmul
    with R as the stationary operand.
  - One AllToAll redistributes R from batch-sharding to k-sharding (1 MB/core).
  - Final einsum s[b,k,d] = sum_{n,e} W[n,k,d,e] R[b,n,k] x[b,n,e] is
    k-sharded (4 output capsules per core, W slice 8.4 MB). Contraction over n
    in chunks of 128, one accumulating matmul per (e, chunk); the moving
    operand y = R*x is built by DVE with free-dim broadcasts only.
  - Each core computes its own output k-slice; host concatenates.
"""

import numpy as np

import concourse.bass as bass
import concourse.bacc as bacc
import concourse.tile as tile
from concourse import mybir
from concourse.bass_utils import run_bass_kernel_spmd

F32 = mybir.dt.float32
B, N, K, D = 32, 2048, 32, 16
NC, BL, KL = 8, 4, 4
NCH = N // 128           # 16 n-chunks
LOG2PI = float(np.log(2.0 * np.pi))
EPS = 1e-7
ROUTINGS = 3
SLAB = 4                 # chunks per softmax slab (FD = 512)


def _ap(ref, offset_elems, dims):
    """Build an AP on ref's tensor with explicit [step, count] dims (elements)."""
    return bass.AP(tensor=ref.tensor, offset=ref.offset + offset_elems, ap=list(dims))


def _build_program():
    nc = bacc.Bacc("TRN2", target_bir_lowering=False)

    xstack = nc.dram_tensor("xstack", [128, N], F32, kind="ExternalInput")
    xmstep = nc.dram_tensor("xmstep", [N, 129], F32, kind="ExternalInput")
    coef0 = nc.dram_tensor("coef0", [128, 128], F32, kind="ExternalInput")
    ccr0 = nc.dram_tensor("ccr0", [128, 128], F32, kind="ExternalInput")
    xfe = nc.dram_tensor("xfe", [N, 16, 32], F32, kind="ExternalInput")
    wstack = nc.dram_tensor("wstack", [16, NCH, 128, 64], F32, kind="ExternalInput")
    ident = nc.dram_tensor("ident", [128, 128], F32, kind="ExternalInput")
    out_t = nc.dram_tensor("out_t", [32, 64], F32, kind="ExternalOutput")

    with tile.TileContext(nc) as tc:
        with (
            tc.tile_pool(name="per", bufs=1) as per,
            tc.tile_pool(name="cf", bufs=1) as cfpool,
            tc.tile_pool(name="soft", bufs=1) as soft,
            tc.tile_pool(name="sm", bufs=4) as sm,
            tc.tile_pool(name="yt", bufs=3) as ytpool,
            tc.tile_pool(name="lg", bufs=2, space="PSUM") as lgpool,
            tc.tile_pool(name="msps", bufs=1, space="PSUM") as mspool,
            tc.tile_pool(name="tps", bufs=3, space="PSUM") as tps,
            tc.tile_pool(name="feps", bufs=1, space="PSUM") as feps,
            tc.tile_pool(name="dram", bufs=1, space="DRAM") as dram,
        ):
            # ---------- persistent loads ----------
            xs = per.tile([128, N], F32)
            for ch in range(NCH):
                nc.sync.dma_start(out=xs[:, ch * 128:(ch + 1) * 128],
                                  in_=xstack[:, ch * 128:(ch + 1) * 128])
            xm = per.tile([128, NCH, 129], F32)
            xmr = xmstep[:].rearrange("(c p) f -> p c f", p=128)
            for ch in range(NCH):
                nc.sync.dma_start(out=xm[:, ch, :], in_=xmr[:, ch, :])
            idt = per.tile([128, 128], F32)
            nc.sync.dma_start(out=idt, in_=ident[:])
            cfb_in = per.tile([128, 128], F32)
            nc.sync.dma_start(out=cfb_in, in_=coef0[:])
            ccr_in = per.tile([128, 128], F32)
            nc.sync.dma_start(out=ccr_in, in_=ccr0[:])
            xf = per.tile([128, NCH, 16, 32], F32)
            xfr = xfe[:].rearrange("(c p) e b -> p c e b", p=128)
            for ch in range(NCH):
                nc.sync.dma_start(out=xf[:, ch], in_=xfr[:, ch])
            wst = per.tile([128, 16, NCH, 64], F32)
            wsr = wstack[:].rearrange("e c p f -> p e c f")
            for ch in range(NCH):
                nc.sync.dma_start(out=wst[:, :, ch, :], in_=wsr[:, :, ch, :])
            ones1 = per.tile([1, 128], F32)
            nc.vector.memset(ones1, 1.0)

            cfb_work = cfpool.tile([128, 128], F32)
            nc.gpsimd.memset(cfb_work, 0.0)
            ccr_work = cfpool.tile([128, 128], F32)

            Ptil = soft.tile([128, NCH, 128], F32)   # exp(logits+cconst)
            lgC = soft.tile([128, NCH, 128], F32)    # logits + cconst
            Zr = soft.tile([128, NCH, 4], F32)       # 1/Z per (n', ch, b)
            Rbuf = soft.tile([128, NCH, 128], F32)   # R, (b,k) col order
            Rlast = soft.tile([128, NCH, 128], F32)  # R final, (k,b)->4k+b col order

            R_loc = dram.tile([8, N, 16], F32)
            RGa = dram.tile([8, N, 16], F32)

            cc_C = float(np.exp(-np.log(float(N)) - (D / 2.0) * LOG2PI))

            for it in range(ROUTINGS):
                cfb = cfb_in if it == 0 else cfb_work
                ccr = ccr_in if it == 0 else ccr_work
                last = it == ROUTINGS - 1

                for s in range(NCH // SLAB):
                    chs = slice(s * SLAB, (s + 1) * SLAB)
                    lg_ps = lgpool.tile([128, SLAB, 128], F32)
                    for i in range(SLAB):
                        ch = s * SLAB + i
                        nc.tensor.matmul(
                            lg_ps[:, i, :],
                            xs[:, ch * 128:(ch + 1) * 128],
                            cfb,
                            start=True,
                            stop=True,
                        )
                    # evict logits PSUM->SBUF on ACT, add cconst on GPSIMD
                    # (keeps DVE free for the reduce/normalize chain)
                    lgE = sm.tile([128, SLAB, 128], F32)
                    nc.scalar.copy(lgE, lg_ps)
                    h = SLAB // 2
                    nc.vector.tensor_add(
                        lgC[:, s * SLAB:s * SLAB + h, :],
                        lgE[:, 0:h],
                        _ap(ccr[:], 0, [[128, 128], [0, h], [1, 128]]),
                    )
                    nc.gpsimd.tensor_add(
                        lgC[:, s * SLAB + h:(s + 1) * SLAB, :],
                        lgE[:, h:SLAB],
                        _ap(ccr[:], 0, [[128, 128], [0, SLAB - h], [1, 128]]),
                    )
                    nc.scalar.activation(
                        Ptil[:, chs, :], lgC[:, chs, :],
                        mybir.ActivationFunctionType.Exp,
                    )
                    # Z over k segments: view [p, (c b), k]
                    zt = sm.tile([128, SLAB, 4], F32)
                    nc.vector.reduce_sum(
                        zt.rearrange("p c b -> p (c b)"),
                        Ptil[:, chs, :].rearrange("p c (b k) -> p (c b) k", b=4),
                        axis=mybir.AxisListType.X,
                    )
                    nc.vector.reciprocal(Zr[:, chs, :].rearrange("p c b -> p (c b)"),
                                         zt.rearrange("p c b -> p (c b)"))
                    for eng, c0, c1 in ((nc.vector, 0, 3), (nc.gpsimd, 3, SLAB)):
                        sub = slice(s * SLAB + c0, s * SLAB + c1)
                        zr_b = _ap(Zr[:, sub, :], 0,
                                   [[NCH * 4, 128], [4, c1 - c0], [1, 4], [0, K]])
                        if not last:
                            # R in (b,k) col order for the M-step
                            eng.tensor_mul(
                                Rbuf[:, sub, :].rearrange("p c (b k) -> p c b k", b=4),
                                Ptil[:, sub, :].rearrange("p c (b k) -> p c b k", b=4),
                                zr_b,
                            )
                        else:
                            # final R written with permuted cols: col' = 4k + b
                            eng.tensor_mul(
                                Rlast[:, sub, :].rearrange("p c (k b) -> p c b k", k=K),
                                Ptil[:, sub, :].rearrange("p c (b k) -> p c b k", b=4),
                                zr_b,
                            )

                if last:
                    break

                # ---------- M-step ----------
                ms_ps = mspool.tile([128, 129], F32)
                for ch in range(NCH):
                    nc.tensor.matmul(
                        ms_ps, Rbuf[:, ch, :], xm[:, ch, :],
                        start=(ch == 0), stop=(ch == NCH - 1),
                    )

                rNk = sm.tile([128, 1], F32)
                nc.vector.reciprocal(rNk, ms_ps[:, 128:129])
                SxA = sm.tile([128, 16], F32)
                SxxA = sm.tile([128, 16], F32)
                for b in range(BL):
                    rows = slice(32 * b, 32 * b + 32)
                    nc.scalar.copy(SxA[rows, :], ms_ps[rows, 16 * b:16 * b + 16])
                    nc.scalar.copy(SxxA[rows, :], ms_ps[rows, 64 + 16 * b:64 + 16 * b + 16])
                mu = sm.tile([128, 16], F32)
                nc.vector.tensor_mul(mu, SxA, rNk.to_broadcast([128, 16]))
                ex2 = sm.tile([128, 16], F32)
                nc.vector.tensor_mul(ex2, SxxA, rNk.to_broadcast([128, 16]))
                musq = sm.tile([128, 16], F32)
                nc.vector.tensor_mul(musq, mu, mu)
                sig2 = sm.tile([128, 16], F32)
                nc.vector.tensor_sub(sig2, ex2, musq)
                a_r = sm.tile([128, 16], F32)
                nc.vector.reciprocal(a_r, sig2)
                bco = sm.tile([128, 16], F32)
                nc.vector.tensor_mul(bco, mu, a_r)

                coefsrc = sm.tile([128, 32], F32)
                nc.scalar.activation(coefsrc[:, 0:16], a_r,
                                     mybir.ActivationFunctionType.Copy, scale=-0.5)
                nc.scalar.copy(coefsrc[:, 16:32], bco)

                lns = sm.tile([128, 16], F32)
                nc.scalar.activation(lns, sig2, mybir.ActivationFunctionType.Ln)
                s1 = sm.tile([128, 1], F32)
                nc.vector.reduce_sum(s1, lns, axis=mybir.AxisListType.X)
                mb = sm.tile([128, 16], F32)
                nc.vector.tensor_mul(mb, mu, bco)
                s2 = sm.tile([128, 1], F32)
                nc.vector.reduce_sum(s2, mb, axis=mybir.AxisListType.X)
                s12 = sm.tile([128, 1], F32)
                nc.vector.tensor_add(s12, s1, s2)
                lnNkc = sm.tile([128, 1], F32)
                nc.scalar.activation(lnNkc, ms_ps[:, 128:129],
                                     mybir.ActivationFunctionType.Ln, scale=cc_C)
                cc = sm.tile([128, 1], F32)
                nc.scalar.activation(cc, s12, mybir.ActivationFunctionType.Identity,
                                     bias=lnNkc, scale=-0.5)

                # coefblock diagonal: transpose [128(b,k), 32(2d)] -> [32(2d), 128(b,k)]
                # then DMA each b's block onto the diagonal (DMA remaps partitions).
                cfT_ps = tps.tile([32, 128], F32, tag="tp")
                nc.tensor.transpose(cfT_ps, coefsrc, idt)
                cfT_sb = sm.tile([32, 128], F32)
                nc.scalar.copy(cfT_sb, cfT_ps)
                for b in range(BL):
                    rows = slice(32 * b, 32 * b + 32)
                    nc.gpsimd.dma_start(
                        out=cfb_work[rows, 32 * b:32 * b + 32],
                        in_=cfT_sb[:, rows],
                    )

                # cconst replicated: transpose [128,1]->[1,128], then ones-outer-product
                ccrow_ps = tps.tile([1, 128], F32, tag="tp")
                nc.tensor.transpose(ccrow_ps, cc, idt)
                ccrowS = sm.tile([1, 128], F32)
                nc.scalar.copy(ccrowS, ccrow_ps)
                ccr_ps = tps.tile([128, 128], F32, tag="tp")
                nc.tensor.matmul(ccr_ps, ones1, ccrowS, start=True, stop=True)
                nc.scalar.copy(ccr_work, ccr_ps)

            # ---------- redistribute R: batch-shard -> k-shard ----------
            for g in range(8):
                nc.sync.dma_start(
                    out=R_loc[g].rearrange("(c p) f -> p c f", p=128),
                    in_=Rlast[:, :, 16 * g:16 * g + 16],
                )
            nc.gpsimd.collective_compute(
                "AllToAll",
                mybir.AluOpType.bypass,
                replica_groups=[list(range(NC))],
                ins=[R_loc[:].opt()],
                outs=[RGa[:].opt()],
            )
            # Rfe[p, ch, kk, b] with b = 4*r + bl contiguous (32 cols per kk)
            Rfe = per.tile([128, NCH, 4, 32], F32)
            for r in range(8):
                rga_r = RGa[r].rearrange("(c p) (kk bl) -> p c kk bl", p=128, kk=4)
                for kk in range(4):
                    nc.sync.dma_start(
                        out=Rfe[:, :, kk, 4 * r:4 * r + 4],
                        in_=rga_r[:, :, kk, :],
                    )

            # ---------- final einsum, k-sharded ----------
            s_ps = feps.tile([64, 128], F32)
            for ch in range(NCH):
                yt = ytpool.tile([128, 16, 4, 8, 4], F32)
                # y[p,(e,kk,r,bl)] = R[p,(kk,r,bl)] * x[p,(e,b)]
                # split the y build across DVE (e<10) and GPSIMD (e>=10):
                # GPSIMD runs ~2x slower per element, so it gets 6/16 of e.
                ytv = yt.rearrange("p e kk r bl -> p e kk (r bl)")
                for eng, e0, e1 in ((nc.vector, 0, 11), (nc.gpsimd, 11, 16)):
                    ecnt = e1 - e0
                    eng.tensor_mul(
                        ytv[:, e0:e1],
                        _ap(Rfe[:, ch], 0,
                            [[NCH * 4 * 32, 128], [0, ecnt], [32, 4], [1, 32]]),
                        _ap(xf[:, ch], 32 * e0,
                            [[NCH * 16 * 32, 128], [32, ecnt], [0, 4], [1, 32]]),
                    )
                for e in range(16):
                    nc.tensor.matmul(
                        s_ps,
                        wst[:, e, ch, :],
                        yt[:, e].rearrange("p a b c -> p (a b c)"),
                        start=(ch == 0 and e == 0),
                        stop=(ch == NCH - 1 and e == 15),
                    )

            # ---------- squash + output ----------
            s_sb = sm.tile([64, 128], F32)
            nc.scalar.copy(s_sb, s_ps)
            sq = sm.tile([64, 32], F32)
            for kk in range(KL):
                rows = slice(16 * kk, 16 * kk + 16)
                nc.gpsimd.dma_start(out=sq[rows, :],
                                     in_=s_sb[rows, 32 * kk:32 * kk + 32])
            sqT_ps = tps.tile([32, 64], F32, tag="tp")
            nc.tensor.transpose(sqT_ps, sq, idt[0:64, 0:64])
            sqT = sm.tile([32, 64], F32)
            nc.scalar.copy(sqT, sqT_ps)
            ssq = sm.tile([32, 64], F32)
            nc.vector.tensor_mul(ssq, sqT, sqT)
            ss = sm.tile([32, 4], F32)
            nc.vector.reduce_sum(ss, ssq.rearrange("p (kk d) -> p kk d", kk=4),
                                 axis=mybir.AxisListType.X)
            eps_t = sm.tile([32, 1], F32)
            nc.vector.memset(eps_t, EPS)
            onep_t = sm.tile([32, 1], F32)
            nc.vector.memset(onep_t, 1.0 + EPS)
            nrm = sm.tile([32, 4], F32)
            nc.scalar.activation(nrm, ss, mybir.ActivationFunctionType.Sqrt, bias=eps_t)
            den = sm.tile([32, 4], F32)
            nc.scalar.activation(den, ss, mybir.ActivationFunctionType.Identity,
                                 bias=onep_t)
            rden = sm.tile([32, 4], F32)
            nc.vector.reciprocal(rden, den)
            fac = sm.tile([32, 4], F32)
            nc.vector.tensor_mul(fac, nrm, rden)
            outT = sm.tile([32, 64], F32)
            nc.vector.tensor_mul(
                outT.rearrange("p (kk d) -> p kk d", kk=4),
                sqT.rearrange("p (kk d) -> p kk d", kk=4),
                _ap(fac[:], 0, [[4, 32], [1, 4], [0, 16]]),
            )
            nc.sync.dma_start(out=out_t[:], in_=outT)

    nc.finalize()
    return nc


_PROG = None


def _get_program():
    global _PROG
    if _PROG is None:
        _PROG = _build_program()
    return _PROG


def _stage_inputs(x, W, mu0):
    x = np.asarray(x, np.float32)
    W = np.asarray(W, np.float32)
    mu0 = np.asarray(mu0, np.float32)
    I128 = np.eye(128, dtype=np.float32)
    # xfe[n, e, b] = x[b, n, e]  (global, shared by all cores)
    xfe = np.ascontiguousarray(x.transpose(1, 2, 0))

    in_maps = []
    for c in range(NC):
        xl = x[BL * c:BL * c + BL]          # [4, N, D]
        x2l = xl * xl
        mul = mu0[BL * c:BL * c + BL]       # [4, K, D]

        xstack = np.empty((128, N), np.float32)
        for b in range(BL):
            xstack[32 * b:32 * b + 16] = x2l[b].T
            xstack[32 * b + 16:32 * b + 32] = xl[b].T

        xmstep = np.empty((N, 129), np.float32)
        for b in range(BL):
            xmstep[:, 16 * b:16 * b + 16] = xl[b]
            xmstep[:, 64 + 16 * b:64 + 16 * b + 16] = x2l[b]
        xmstep[:, 128] = 1.0

        coef0 = np.zeros((128, 128), np.float32)
        for b in range(BL):
            coef0[32 * b:32 * b + 16, 32 * b:32 * b + 32] = -0.5
            coef0[32 * b + 16:32 * b + 32, 32 * b:32 * b + 32] = mul[b].T
        cconst = (np.log(1.0 / K)
                  - 0.5 * ((mul * mul).sum(-1) + D * LOG2PI))  # [4, K]
        ccr0 = np.broadcast_to(cconst.reshape(1, 128), (128, 128)).astype(np.float32)
        ccr0 = np.ascontiguousarray(ccr0)

        # wstack[e, ch, n', 16*kk+dd] = W[0, 128*ch+n', 4c+kk, dd, e]
        Wl = W[0][:, KL * c:KL * c + KL]    # [N, 4, D, E]
        wstack = np.ascontiguousarray(
            Wl.reshape(NCH, 128, KL, D, D).transpose(4, 0, 1, 2, 3)
            .reshape(16, NCH, 128, 64))

        in_maps.append({
            "xstack": xstack, "xmstep": xmstep, "coef0": coef0, "ccr0": ccr0,
            "xfe": xfe, "wstack": wstack, "ident": I128,
        })
    return in_maps


_RUNNER = None


def _get_runner():
    """Build (once) an SPMD executor mirroring bass2jax.run_bass_via_pjrt.

    Compiled via bass2jax.fast_dispatch_compile: the bass_effect is
    suppressed so calls take jax's C++ fast dispatch path instead of the
    Python effects path (lower per-call host overhead)."""
    global _RUNNER
    if _RUNNER is not None:
        return _RUNNER
    import jax
    from jax.sharding import Mesh, PartitionSpec, NamedSharding
    from jax.experimental.shard_map import shard_map
    from concourse import bass2jax, mybir as _mb

    nc = _get_program()
    bass2jax.install_neuronx_cc_hook()
    partition_name = nc.partition_id_tensor.name if nc.partition_id_tensor else None
    in_names, out_names, out_avals, zero_outs = [], [], [], []
    for alloc in nc.m.functions[0].allocations:
        if not isinstance(alloc, _mb.MemoryLocationSet):
            continue
        name = alloc.memorylocations[0].name
        if alloc.kind == "ExternalInput":
            if name != partition_name:
                in_names.append(name)
        elif alloc.kind == "ExternalOutput":
            shape = tuple(alloc.tensor_shape)
            dtype = _mb.dt.np(alloc.dtype)
            out_names.append(name)
            out_avals.append(jax.core.ShapedArray(shape, dtype))
            zero_outs.append(np.zeros(shape, dtype))
    n_params = len(in_names)
    n_outs = len(out_avals)
    all_names = in_names + out_names
    if partition_name is not None:
        all_names.append(partition_name)

    def _body(*args):
        operands = list(args)
        if partition_name is not None:
            operands.append(bass2jax.partition_id_tensor())
        outs = bass2jax._bass_exec_p.bind(
            *operands,
            out_avals=tuple(out_avals),
            in_names=tuple(all_names),
            out_names=tuple(out_names),
            lowering_input_output_aliases=(),
            sim_require_finite=True,
            sim_require_nnan=True,
            nc=nc,
        )
        return tuple(outs)

    global _MESH
    devices = jax.devices()[:NC]
    mesh = Mesh(np.asarray(devices), ("core",))
    _MESH = mesh
    in_specs = (PartitionSpec("core"),) * (n_params + n_outs)
    out_specs = (PartitionSpec("core"),) * n_outs
    shd = NamedSharding(mesh, PartitionSpec("core"))
    state = {}

    def _get_fast(dev_in, dev_zero):
        if "fast" not in state:
            def compile_fn():
                fresh = jax.jit(
                    shard_map(_body, mesh=mesh, in_specs=in_specs,
                              out_specs=out_specs, check_rep=False),
                    keep_unused=True,
                )
                return fresh.lower(*dev_in, *dev_zero).compile()
            state["fast"] = bass2jax.fast_dispatch_compile(compile_fn)
        return state["fast"]

    def run(in_maps):
        concat_in = [
            np.concatenate([np.asarray(in_maps[c][n]) for c in range(NC)], axis=0)
            for n in in_names
        ]
        concat_zeros = [
            np.zeros((NC * z.shape[0], *z.shape[1:]), z.dtype) for z in zero_outs
        ]
        dev_in = [jax.device_put(a, shd) for a in concat_in]
        dev_zero = [jax.device_put(z, shd) for z in concat_zeros]
        fast = _get_fast(dev_in, dev_zero)
        out_arrs = fast(*dev_in, *dev_zero)
        run._sharded = (fast, (in_names, zero_outs, NC))
        return [
            {n: np.asarray(out_arrs[i]).reshape(NC, *out_avals[i].shape)[c]
             for i, n in enumerate(out_names)}
            for c in range(NC)
        ]

    run._sharded = None
    _RUNNER = run
    return run


def kernel(x, W, mu0):
    run = _get_runner()
    in_maps = _stage_inputs(x, W, mu0)
    results = run(in_maps)
    out = np.empty((B, K, D), np.float32)
    for c in range(NC):
        out[:, KL * c:KL * c + KL, :] = results[c]["out_t"].reshape(B, KL, D)
    kernel._last_in_maps = in_maps
    kernel._last_run = run
    kernel._last_sharded = run._sharded
    return out

